# revision 9
# baseline (speedup 1.0000x reference)
"""Contrastive loss (SimCLR-style) on 8 TRN2 NeuronCores.

loss = -mean(diag(log_softmax(zi_n @ zj_n^T / T)))  with zi_n, zj_n L2-normalized,
N=4096, D=256, T=0.5.

Strategy (data-parallel over rows of z_i, z_j replicated):
  - core c gets rows [c*512, (c+1)*512) of z_i, the full z_j, and z_j's
    matching diagonal block as a separate small input.
  - cast to bf16 during load; row norms as one big multiply + one
    reduce per group; rsqrt on VectorE (bit-trick + 1 Newton step) so
    ScalarE's table set stays pinned to exp; row scaling on GpSimd (otherwise
    idle); one 3D-output xbar DMA transpose per group (SBUF->SBUF, no DRAM
    bounce); matmul in bf16 with f32 PSUM accumulate; fused exp+row-sum on
    ScalarE in place over PSUM (logits in [-2,2]: no max subtraction);
    diagonal via fused multiply+accumulate in normal layout; ones-matmul
    partition reduction.
  - z_j is processed in 4 pipelined groups; the logits loop runs
    half-m-range-outer so compute on groups 0-1 overlaps preprocessing of
    groups 2-3.
  - each core returns 4 partial sums of (lse[n] - logits[n,n]); host adds the
    32 values and divides by N.
"""

import numpy as np

import concourse.bass as bass
import concourse.bacc as bacc
import concourse.tile as tile
import concourse.bass_utils as bass_utils
from concourse import mybir

N = 4096
D = 256
NCORES = 8
NL = N // NCORES  # 512 local rows per core
P = 128
NCHUNK = NL // P  # 4 local row chunks
MCHUNK = N // P  # 32 zj chunks
NGROUP = 4  # zj processed in 4 groups of 8 chunks
GCH = MCHUNK // NGROUP  # 8 chunks per group
GM = GCH * P  # 1024 rows per group
KH = D // P  # 2 contraction halves
MAGIC = 0x5F3759DF

F32 = mybir.dt.float32
U32 = mybir.dt.uint32
BF16 = mybir.dt.bfloat16
AF = mybir.ActivationFunctionType
ALU = mybir.AluOpType
AX = mybir.AxisListType


def build_nc():
    nc = bacc.Bacc(
        "TRN2",
        target_bir_lowering=False,
        debug=False,
        enable_asserts=False,
    )
    z_i = nc.dram_tensor("z_i", (NL, D), F32, kind="ExternalInput").ap()
    z_j = nc.dram_tensor("z_j", (N, D), F32, kind="ExternalInput").ap()
    z_jd = nc.dram_tensor("z_jd", (NL, D), F32, kind="ExternalInput").ap()
    out = nc.dram_tensor("out", (1, NCHUNK), F32, kind="ExternalOutput").ap()

    with tile.TileContext(nc) as tc:
        with (
            tc.tile_pool(name="const", bufs=1) as const,
            tc.tile_pool(name="big", bufs=1) as big,
            tc.tile_pool(name="work", bufs=2) as work,
            tc.tile_pool(name="stat", bufs=1) as stat,
            tc.tile_pool(name="psum", bufs=2, space="PSUM") as psum,
        ):
            # --- dummy exp: force the exp ACT table set load at t=0
            dummy = const.tile([1, 1], F32)
            nc.vector.memset(dummy, 1.0)
            nc.scalar.activation(out=dummy, in_=dummy, func=AF.Exp)

            ones = const.tile([P, 1], F32)
            nc.vector.memset(ones, 1.0)
            magic = const.tile([P, GCH], U32)
            nc.vector.memset(magic, MAGIC)

            def rsqrt_dve(a, y, w):
                """y[:,:w] = 1/sqrt(a[:,:w]): quake seed + 1 Newton step."""
                au = a.bitcast(U32)
                yu = y.bitcast(U32)
                sh = work.tile([P, GCH], U32, tag="rsq_sh")
                nc.vector.tensor_scalar(
                    out=sh[:, :w], in0=au, scalar1=1, scalar2=None,
                    op0=ALU.logical_shift_right,
                )
                nc.vector.tensor_sub(out=yu, in0=magic[:, :w], in1=sh[:, :w])
                t1 = work.tile([P, GCH], F32, tag="rsq_t1")
                nc.vector.tensor_mul(out=t1[:, :w], in0=y, in1=y)
                nc.vector.tensor_mul(out=t1[:, :w], in0=t1[:, :w], in1=a)
                nc.vector.tensor_scalar(
                    out=t1[:, :w], in0=t1[:, :w], scalar1=-0.5, scalar2=1.5,
                    op0=ALU.mult, op1=ALU.add,
                )
                nc.vector.tensor_mul(out=y, in0=y, in1=t1[:, :w])

            # --- zi: load (cast bf16), transpose immediately, then norms
            zi_bf = big.tile([P, NCHUNK, D], BF16)
            nc.gpsimd.dma_start(
                out=zi_bf, in_=z_i.rearrange("(c p) d -> p c d", p=P)
            )
            ziT = big.tile([P, NCHUNK * KH, P], BF16)
            nc.sync.dma_start_transpose(
                out=ziT, in_=zi_bf.rearrange("p c d -> p (c d)")
            )
            ziT_r = ziT.rearrange("do (i h) m -> do i h m", h=KH)

            sq_i = work.tile([P, NCHUNK, D], BF16, tag="sqi")
            nc.vector.tensor_mul(out=sq_i, in0=zi_bf, in1=zi_bf)
            nrm2_i = stat.tile([P, NCHUNK], F32)
            nc.vector.tensor_reduce(out=nrm2_i, in_=sq_i, axis=AX.X, op=ALU.add)
            s2 = stat.tile([P, NCHUNK], F32)
            rsqrt_dve(nrm2_i, s2, NCHUNK)
            nc.vector.tensor_scalar(
                out=s2, in0=s2, scalar1=2.0, scalar2=None, op0=ALU.mult
            )

            # --- per-group zj: load -> norms -> rsqrt -> scale (GpSimd) ->
            #     one 3D xbar transpose
            nrm2_j = stat.tile([P, MCHUNK], F32)
            t_j = stat.tile([P, MCHUNK], F32)
            zjT_r = []

            def zj_group(g):
                zj_bf = big.tile([P, GCH, D], BF16, tag=f"zjbf{g}")
                nc.gpsimd.dma_start(
                    out=zj_bf,
                    in_=z_j[g * GM : (g + 1) * GM, :].rearrange(
                        "(c p) d -> p c d", p=P
                    ),
                )
                sq = work.tile([P, GCH, D], BF16, tag="sqg")
                nc.vector.tensor_mul(out=sq, in0=zj_bf, in1=zj_bf)
                gs = slice(g * GCH, (g + 1) * GCH)
                nc.vector.tensor_reduce(
                    out=nrm2_j[:, gs], in_=sq, axis=AX.X, op=ALU.add
                )
                rsqrt_dve(nrm2_j[:, gs], t_j[:, gs], GCH)
                zjs = big.tile([P, GCH, D], BF16, tag=f"zjs{g}")
                for jl in range(GCH):
                    j = g * GCH + jl
                    nc.gpsimd.tensor_scalar_mul(
                        out=zjs[:, jl, :],
                        in0=zj_bf[:, jl, :],
                        scalar1=t_j[:, j : j + 1],
                    )
                zjT = big.tile([P, GCH * KH, P], BF16, tag=f"zjT{g}")
                eng = nc.sync if g % 2 == 0 else nc.scalar
                eng.dma_start_transpose(
                    out=zjT, in_=zjs.rearrange("p c d -> p (c d)")
                )
                zjT_r.append(zjT.rearrange("do (c h) m -> do c h m", h=KH))

            # --- main compute: one [128, 2048] logits tile + fused exp
            MW = 2048
            NSL = MW // 512
            lse_parts = stat.tile([P, 2, NCHUNK], F32)

            def logits_tile(i, half):
                pt = psum.tile([P, MW], F32, tag="pt")
                for h in range(KH):
                    for jj in range(NSL):
                        m0 = half * MW + jj * 512
                        g = m0 // GM
                        c0 = (m0 % GM) // P
                        nc.tensor.matmul(
                            pt[:, jj * 512 : (jj + 1) * 512],
                            lhsT=ziT_r[:, i, h, :],
                            rhs=zjT_r[g][:, c0 : c0 + 4, h, :],
                            start=(h == 0),
                            stop=(h == KH - 1),
                        )
                nc.scalar.activation(
                    out=pt,
                    in_=pt,
                    func=AF.Exp,
                    scale=s2[:, i : i + 1],
                    accum_out=lse_parts[:, half, i : i + 1],
                )

            # pipeline: groups 0-1, half-0 tiles overlap groups 2-3 preprocessing
            zj_group(0)
            zj_group(1)
            for i in range(NCHUNK):
                logits_tile(i, 0)
            zj_group(2)
            zj_group(3)
            for i in range(NCHUNK):
                logits_tile(i, 1)

            # --- diagonal block (loaded late; needed only at the end)
            zjd_bf = big.tile([P, NCHUNK, D], BF16)
            nc.gpsimd.dma_start(
                out=zjd_bf, in_=z_jd.rearrange("(c p) d -> p c d", p=P)
            )
            sq_d = work.tile([P, NCHUNK, D], BF16, tag="sqi")
            nc.vector.tensor_mul(out=sq_d, in0=zjd_bf, in1=zjd_bf)
            nrm2_d = stat.tile([P, NCHUNK], F32)
            nc.vector.tensor_reduce(out=nrm2_d, in_=sq_d, axis=AX.X, op=ALU.add)
            t_d = stat.tile([P, NCHUNK], F32)
            rsqrt_dve(nrm2_d, t_d, NCHUNK)
            zjds = big.tile([P, NCHUNK, D], BF16)
            for i in range(NCHUNK):
                nc.gpsimd.tensor_scalar_mul(
                    out=zjds[:, i, :], in0=zjd_bf[:, i, :], scalar1=t_d[:, i : i + 1]
                )
            dt = stat.tile([P, NCHUNK], F32)
            for i in range(NCHUNK):
                sq = work.tile([P, D], BF16, tag="sq")
                nc.vector.scalar_tensor_tensor(
                    out=sq, in0=zi_bf[:, i, :], scalar=1.0, in1=zjds[:, i, :],
                    op0=ALU.mult, op1=ALU.mult,
                    accum_out=dt[:, i : i + 1],
                )
            dg = stat.tile([P, NCHUNK], F32)
            nc.vector.tensor_mul(out=dg, in0=dt, in1=s2)

            # --- lse = ln(sum of the two half row-sums); contrib = lse - diag
            rs = stat.tile([P, NCHUNK], F32)
            nc.vector.tensor_add(
                out=rs, in0=lse_parts[:, 0, :], in1=lse_parts[:, 1, :]
            )
            lse = stat.tile([P, NCHUNK], F32)
            nc.scalar.activation(out=lse, in_=rs, func=AF.Ln)
            contrib = stat.tile([P, NCHUNK], F32)
            nc.vector.tensor_sub(out=contrib, in0=lse, in1=dg)

            # --- partition reduction via ones-matmul: [1, 4] partials
            pt_fin = psum.tile([P, MW], F32, tag="pt")
            nc.tensor.matmul(
                pt_fin[:1, :NCHUNK], lhsT=ones, rhs=contrib, start=True, stop=True
            )
            osb = stat.tile([1, NCHUNK], F32)
            nc.vector.tensor_copy(out=osb, in_=pt_fin[:1, :NCHUNK])
            nc.sync.dma_start(out=out, in_=osb)

    nc.compile()
    return nc


_NC = None


def _get_nc():
    global _NC
    if _NC is None:
        _NC = build_nc()
    return _NC


def kernel(z_i: np.ndarray, z_j: np.ndarray, **_unused) -> np.ndarray:
    z_i = np.ascontiguousarray(z_i, dtype=np.float32)
    z_j = np.ascontiguousarray(z_j, dtype=np.float32)
    nc = _get_nc()
    in_maps = []
    for c in range(NCORES):
        sl = slice(c * NL, (c + 1) * NL)
        in_maps.append(
            {
                "z_i": z_i[sl],
                "z_j": z_j,
                "z_jd": z_j[sl],
            }
        )
    res = bass_utils.run_bass_kernel_spmd(
        nc, in_maps, core_ids=list(range(NCORES))
    )
    total = 0.0
    for c in range(NCORES):
        total += float(res.results[c]["out"].astype(np.float64).sum())
    return np.float32(total / N)


# revision 10
# speedup vs baseline: 2.5238x; 2.5238x over previous
"""Contrastive loss (SimCLR-style) on 8 TRN2 NeuronCores.

loss = -mean(diag(log_softmax(zi_n @ zj_n^T / T)))  with zi_n, zj_n L2-normalized,
N=4096, D=256, T=0.5.

Strategy (data-parallel over rows of z_i, z_j replicated):
  - core c gets rows [c*512, (c+1)*512) of z_i, the full z_j, and z_j's
    matching diagonal block as a separate small input.
  - cast to bf16 during load; row norms as one big multiply + one
    reduce per group; rsqrt on VectorE (bit-trick + 1 Newton step) so
    ScalarE's table set stays pinned to exp; row scaling on GpSimd (otherwise
    idle); one 3D-output xbar DMA transpose per group (SBUF->SBUF, no DRAM
    bounce); matmul in bf16 with f32 PSUM accumulate; fused exp+row-sum on
    ScalarE in place over PSUM (logits in [-2,2]: no max subtraction);
    diagonal via fused multiply+accumulate in normal layout; ones-matmul
    partition reduction.
  - z_j is processed in 4 pipelined groups; the logits loop runs
    half-m-range-outer so compute on groups 0-1 overlaps preprocessing of
    groups 2-3.
  - each core returns 4 partial sums of (lse[n] - logits[n,n]); host adds the
    32 values and divides by N.
"""

import numpy as np

import concourse.bass as bass
import concourse.bacc as bacc
import concourse.tile as tile
import concourse.bass_utils as bass_utils
from concourse import mybir

N = 4096
D = 256
NCORES = 8
NL = N // NCORES  # 512 local rows per core
P = 128
NCHUNK = NL // P  # 4 local row chunks
MCHUNK = N // P  # 32 zj chunks
NGROUP = 4  # zj processed in 4 groups of 8 chunks
GCH = MCHUNK // NGROUP  # 8 chunks per group
GM = GCH * P  # 1024 rows per group
KH = D // P  # 2 contraction halves
MAGIC = 0x5F3759DF

F32 = mybir.dt.float32
U32 = mybir.dt.uint32
BF16 = mybir.dt.bfloat16
AF = mybir.ActivationFunctionType
ALU = mybir.AluOpType
AX = mybir.AxisListType


def build_nc():
    nc = bacc.Bacc(
        "TRN2",
        target_bir_lowering=False,
        debug=False,
        enable_asserts=False,
    )
    z_i = nc.dram_tensor("z_i", (NL, D), F32, kind="ExternalInput").ap()
    z_j = nc.dram_tensor("z_j", (N, D), F32, kind="ExternalInput").ap()
    z_jd = nc.dram_tensor("z_jd", (NL, D), F32, kind="ExternalInput").ap()
    out = nc.dram_tensor("out", (1, NCHUNK), F32, kind="ExternalOutput").ap()

    with tile.TileContext(nc) as tc:
        with (
            tc.tile_pool(name="const", bufs=1) as const,
            tc.tile_pool(name="big", bufs=1) as big,
            tc.tile_pool(name="work", bufs=2) as work,
            tc.tile_pool(name="stat", bufs=1) as stat,
            tc.tile_pool(name="psum", bufs=2, space="PSUM") as psum,
        ):
            # --- dummy exp: force the exp ACT table set load at t=0
            dummy = const.tile([1, 1], F32)
            nc.vector.memset(dummy, 1.0)
            nc.scalar.activation(out=dummy, in_=dummy, func=AF.Exp)

            ones = const.tile([P, 1], F32)
            nc.vector.memset(ones, 1.0)
            magic = const.tile([P, GCH], U32)
            nc.vector.memset(magic, MAGIC)

            def rsqrt_dve(a, y, w):
                """y[:,:w] = 1/sqrt(a[:,:w]): quake seed + 1 Newton step."""
                au = a.bitcast(U32)
                yu = y.bitcast(U32)
                sh = work.tile([P, GCH], U32, tag="rsq_sh")
                nc.vector.tensor_scalar(
                    out=sh[:, :w], in0=au, scalar1=1, scalar2=None,
                    op0=ALU.logical_shift_right,
                )
                nc.vector.tensor_sub(out=yu, in0=magic[:, :w], in1=sh[:, :w])
                t1 = work.tile([P, GCH], F32, tag="rsq_t1")
                nc.vector.tensor_mul(out=t1[:, :w], in0=y, in1=y)
                nc.vector.tensor_mul(out=t1[:, :w], in0=t1[:, :w], in1=a)
                nc.vector.tensor_scalar(
                    out=t1[:, :w], in0=t1[:, :w], scalar1=-0.5, scalar2=1.5,
                    op0=ALU.mult, op1=ALU.add,
                )
                nc.vector.tensor_mul(out=y, in0=y, in1=t1[:, :w])

            # --- zi: load (cast bf16), transpose immediately, then norms
            zi_bf = big.tile([P, NCHUNK, D], BF16)
            nc.gpsimd.dma_start(
                out=zi_bf, in_=z_i.rearrange("(c p) d -> p c d", p=P)
            )
            ziT = big.tile([P, NCHUNK * KH, P], BF16)
            nc.sync.dma_start_transpose(
                out=ziT, in_=zi_bf.rearrange("p c d -> p (c d)")
            )
            ziT_r = ziT.rearrange("do (i h) m -> do i h m", h=KH)

            nrm2_i = stat.tile([P, NCHUNK], F32)
            for i in range(NCHUNK):
                sq = work.tile([P, D], BF16, tag="sq")
                nc.vector.scalar_tensor_tensor(
                    out=sq, in0=zi_bf[:, i, :], scalar=1.0, in1=zi_bf[:, i, :],
                    op0=ALU.mult, op1=ALU.mult,
                    accum_out=nrm2_i[:, i : i + 1],
                )
            s2 = stat.tile([P, NCHUNK], F32)
            rsqrt_dve(nrm2_i, s2, NCHUNK)
            nc.vector.tensor_scalar(
                out=s2, in0=s2, scalar1=2.0, scalar2=None, op0=ALU.mult
            )

            # --- per-group zj: load -> norms -> rsqrt -> scale (GpSimd) ->
            #     one 3D xbar transpose
            nrm2_j = stat.tile([P, MCHUNK], F32)
            t_j = stat.tile([P, MCHUNK], F32)
            zjT_r = []

            def zj_group(g):
                zj_bf = big.tile([P, GCH, D], BF16, tag=f"zjbf{g}")
                nc.gpsimd.dma_start(
                    out=zj_bf,
                    in_=z_j[g * GM : (g + 1) * GM, :].rearrange(
                        "(c p) d -> p c d", p=P
                    ),
                )
                for jl in range(GCH):
                    j = g * GCH + jl
                    sq = work.tile([P, D], BF16, tag="sq")
                    nc.vector.scalar_tensor_tensor(
                        out=sq, in0=zj_bf[:, jl, :], scalar=1.0,
                        in1=zj_bf[:, jl, :],
                        op0=ALU.mult, op1=ALU.mult,
                        accum_out=nrm2_j[:, j : j + 1],
                    )
                gs = slice(g * GCH, (g + 1) * GCH)
                rsqrt_dve(nrm2_j[:, gs], t_j[:, gs], GCH)
                zjs = big.tile([P, GCH, D], BF16, tag=f"zjs{g}")
                for jl in range(GCH):
                    j = g * GCH + jl
                    nc.vector.tensor_scalar_mul(
                        out=zjs[:, jl, :],
                        in0=zj_bf[:, jl, :],
                        scalar1=t_j[:, j : j + 1],
                    )
                zjT = big.tile([P, GCH * KH, P], BF16, tag=f"zjT{g}")
                eng = nc.sync if g % 2 == 0 else nc.scalar
                eng.dma_start_transpose(
                    out=zjT, in_=zjs.rearrange("p c d -> p (c d)")
                )
                zjT_r.append(zjT.rearrange("do (c h) m -> do c h m", h=KH))

            # --- main compute: one [128, 2048] logits tile + fused exp
            MW = 2048
            NSL = MW // 512
            lse_parts = stat.tile([P, 2, NCHUNK], F32)

            def logits_tile(i, half):
                pt = psum.tile([P, MW], F32, tag="pt")
                for h in range(KH):
                    for jj in range(NSL):
                        m0 = half * MW + jj * 512
                        g = m0 // GM
                        c0 = (m0 % GM) // P
                        nc.tensor.matmul(
                            pt[:, jj * 512 : (jj + 1) * 512],
                            lhsT=ziT_r[:, i, h, :],
                            rhs=zjT_r[g][:, c0 : c0 + 4, h, :],
                            start=(h == 0),
                            stop=(h == KH - 1),
                        )
                nc.scalar.activation(
                    out=pt,
                    in_=pt,
                    func=AF.Exp,
                    scale=s2[:, i : i + 1],
                    accum_out=lse_parts[:, half, i : i + 1],
                )

            # pipeline: groups 0-1, half-0 tiles overlap groups 2-3 preprocessing
            zj_group(0)
            zj_group(1)
            for i in range(NCHUNK):
                logits_tile(i, 0)
            zj_group(2)
            zj_group(3)
            for i in range(NCHUNK):
                logits_tile(i, 1)

            # --- diagonal block (loaded late; needed only at the end)
            zjd_bf = big.tile([P, NCHUNK, D], BF16)
            nc.gpsimd.dma_start(
                out=zjd_bf, in_=z_jd.rearrange("(c p) d -> p c d", p=P)
            )
            nrm2_d = stat.tile([P, NCHUNK], F32)
            for i in range(NCHUNK):
                sq = work.tile([P, D], BF16, tag="sq")
                nc.vector.scalar_tensor_tensor(
                    out=sq, in0=zjd_bf[:, i, :], scalar=1.0, in1=zjd_bf[:, i, :],
                    op0=ALU.mult, op1=ALU.mult,
                    accum_out=nrm2_d[:, i : i + 1],
                )
            t_d = stat.tile([P, NCHUNK], F32)
            rsqrt_dve(nrm2_d, t_d, NCHUNK)
            zjds = big.tile([P, NCHUNK, D], BF16)
            for i in range(NCHUNK):
                nc.vector.tensor_scalar_mul(
                    out=zjds[:, i, :], in0=zjd_bf[:, i, :], scalar1=t_d[:, i : i + 1]
                )
            dt = stat.tile([P, NCHUNK], F32)
            for i in range(NCHUNK):
                sq = work.tile([P, D], BF16, tag="sq")
                nc.vector.scalar_tensor_tensor(
                    out=sq, in0=zi_bf[:, i, :], scalar=1.0, in1=zjds[:, i, :],
                    op0=ALU.mult, op1=ALU.mult,
                    accum_out=dt[:, i : i + 1],
                )
            dg = stat.tile([P, NCHUNK], F32)
            nc.vector.tensor_mul(out=dg, in0=dt, in1=s2)

            # --- lse = ln(sum of the two half row-sums); contrib = lse - diag
            rs = stat.tile([P, NCHUNK], F32)
            nc.vector.tensor_add(
                out=rs, in0=lse_parts[:, 0, :], in1=lse_parts[:, 1, :]
            )
            lse = stat.tile([P, NCHUNK], F32)
            nc.scalar.activation(out=lse, in_=rs, func=AF.Ln)
            contrib = stat.tile([P, NCHUNK], F32)
            nc.vector.tensor_sub(out=contrib, in0=lse, in1=dg)

            # --- partition reduction via ones-matmul: [1, 4] partials
            pt_fin = psum.tile([P, MW], F32, tag="pt")
            nc.tensor.matmul(
                pt_fin[:1, :NCHUNK], lhsT=ones, rhs=contrib, start=True, stop=True
            )
            osb = stat.tile([1, NCHUNK], F32)
            nc.vector.tensor_copy(out=osb, in_=pt_fin[:1, :NCHUNK])
            nc.sync.dma_start(out=out, in_=osb)

    nc.compile()
    return nc


_NC = None


def _get_nc():
    global _NC
    if _NC is None:
        _NC = build_nc()
    return _NC


def kernel(z_i: np.ndarray, z_j: np.ndarray, **_unused) -> np.ndarray:
    z_i = np.ascontiguousarray(z_i, dtype=np.float32)
    z_j = np.ascontiguousarray(z_j, dtype=np.float32)
    nc = _get_nc()
    in_maps = []
    for c in range(NCORES):
        sl = slice(c * NL, (c + 1) * NL)
        in_maps.append(
            {
                "z_i": z_i[sl],
                "z_j": z_j,
                "z_jd": z_j[sl],
            }
        )
    res = bass_utils.run_bass_kernel_spmd(
        nc, in_maps, core_ids=list(range(NCORES))
    )
    total = 0.0
    for c in range(NCORES):
        total += float(res.results[c]["out"].astype(np.float64).sum())
    return np.float32(total / N)


# revision 11
# speedup vs baseline: 2.6685x; 1.0573x over previous
"""Contrastive loss (SimCLR-style) on 8 TRN2 NeuronCores.

loss = -mean(diag(log_softmax(zi_n @ zj_n^T / T)))  with zi_n, zj_n L2-normalized,
N=4096, D=256, T=0.5.

Strategy (data-parallel over rows of z_i, z_j replicated):
  - core c gets rows [c*512, (c+1)*512) of z_i, the full z_j, and z_j's
    matching diagonal block as a separate small input.
  - cast to bf16 during load; row norms as one big multiply + one
    reduce per group; rsqrt on VectorE (bit-trick + 1 Newton step) so
    ScalarE's table set stays pinned to exp; row scaling on GpSimd (otherwise
    idle); one 3D-output xbar DMA transpose per group (SBUF->SBUF, no DRAM
    bounce); matmul in bf16 with f32 PSUM accumulate; fused exp+row-sum on
    ScalarE in place over PSUM (logits in [-2,2]: no max subtraction);
    diagonal via fused multiply+accumulate in normal layout; ones-matmul
    partition reduction.
  - z_j is processed in 4 pipelined groups; the logits loop runs
    half-m-range-outer so compute on groups 0-1 overlaps preprocessing of
    groups 2-3.
  - each core returns 4 partial sums of (lse[n] - logits[n,n]); host adds the
    32 values and divides by N.
"""

import numpy as np

import concourse.bass as bass
import concourse.bacc as bacc
import concourse.tile as tile
import concourse.bass_utils as bass_utils
from concourse import mybir

N = 4096
D = 256
NCORES = 8
NL = N // NCORES  # 512 local rows per core
P = 128
NCHUNK = NL // P  # 4 local row chunks
MCHUNK = N // P  # 32 zj chunks
NGROUP = 4  # zj processed in 4 groups of 8 chunks
GCH = MCHUNK // NGROUP  # 8 chunks per group
GM = GCH * P  # 1024 rows per group
KH = D // P  # 2 contraction halves
MAGIC = 0x5F3759DF

F32 = mybir.dt.float32
U32 = mybir.dt.uint32
BF16 = mybir.dt.bfloat16
AF = mybir.ActivationFunctionType
ALU = mybir.AluOpType
AX = mybir.AxisListType


def build_nc():
    nc = bacc.Bacc(
        "TRN2",
        target_bir_lowering=False,
        debug=False,
        enable_asserts=False,
    )
    z_i = nc.dram_tensor("z_i", (NL, D), F32, kind="ExternalInput").ap()
    z_j = nc.dram_tensor("z_j", (N, D), F32, kind="ExternalInput").ap()
    z_jd = nc.dram_tensor("z_jd", (NL, D), F32, kind="ExternalInput").ap()
    out = nc.dram_tensor("out", (1, NCHUNK), F32, kind="ExternalOutput").ap()

    with tile.TileContext(nc) as tc:
        with (
            tc.tile_pool(name="const", bufs=1) as const,
            tc.tile_pool(name="big", bufs=1) as big,
            tc.tile_pool(name="work", bufs=2) as work,
            tc.tile_pool(name="stat", bufs=1) as stat,
            tc.tile_pool(name="psum", bufs=4, space="PSUM") as psum,
        ):
            # --- dummy exp: force the exp ACT table set load at t=0
            dummy = const.tile([1, 1], F32)
            nc.vector.memset(dummy, 1.0)
            nc.scalar.activation(out=dummy, in_=dummy, func=AF.Exp)

            ones = const.tile([P, 1], F32)
            nc.vector.memset(ones, 1.0)
            magic = const.tile([P, GCH], U32)
            nc.vector.memset(magic, MAGIC)

            def rsqrt_dve(a, y, w):
                """y[:,:w] = 1/sqrt(a[:,:w]): quake seed + 1 Newton step."""
                au = a.bitcast(U32)
                yu = y.bitcast(U32)
                sh = work.tile([P, GCH], U32, tag="rsq_sh")
                nc.vector.tensor_scalar(
                    out=sh[:, :w], in0=au, scalar1=1, scalar2=None,
                    op0=ALU.logical_shift_right,
                )
                nc.vector.tensor_sub(out=yu, in0=magic[:, :w], in1=sh[:, :w])
                t1 = work.tile([P, GCH], F32, tag="rsq_t1")
                nc.vector.tensor_mul(out=t1[:, :w], in0=y, in1=y)
                nc.vector.tensor_mul(out=t1[:, :w], in0=t1[:, :w], in1=a)
                nc.vector.tensor_scalar(
                    out=t1[:, :w], in0=t1[:, :w], scalar1=-0.5, scalar2=1.5,
                    op0=ALU.mult, op1=ALU.add,
                )
                nc.vector.tensor_mul(out=y, in0=y, in1=t1[:, :w])

            # --- zi: load (cast bf16), transpose immediately, then norms
            zi_bf = big.tile([P, NCHUNK, D], BF16)
            nc.gpsimd.dma_start(
                out=zi_bf, in_=z_i.rearrange("(c p) d -> p c d", p=P)
            )
            ziT = big.tile([P, NCHUNK * KH, P], BF16)
            nc.scalar.dma_start_transpose(
                out=ziT, in_=zi_bf.rearrange("p c d -> p (c d)")
            )
            ziT_r = ziT.rearrange("do (i h) m -> do i h m", h=KH)

            nrm2_i = stat.tile([P, NCHUNK], F32)
            for i in range(NCHUNK):
                sq = work.tile([P, D], BF16, tag="sq")
                nc.vector.scalar_tensor_tensor(
                    out=sq, in0=zi_bf[:, i, :], scalar=1.0, in1=zi_bf[:, i, :],
                    op0=ALU.mult, op1=ALU.mult,
                    accum_out=nrm2_i[:, i : i + 1],
                )
            s2 = stat.tile([P, NCHUNK], F32)
            rsqrt_dve(nrm2_i, s2, NCHUNK)
            nc.vector.tensor_scalar(
                out=s2, in0=s2, scalar1=2.0, scalar2=None, op0=ALU.mult
            )

            # --- per-group zj: load -> norms -> rsqrt -> scale (GpSimd) ->
            #     one 3D xbar transpose
            nrm2_j = stat.tile([P, MCHUNK], F32)
            t_j = stat.tile([P, MCHUNK], F32)
            zjT_r = []

            def zj_group(g):
                zj_bf = big.tile([P, GCH, D], BF16, tag=f"zjbf{g}")
                nc.gpsimd.dma_start(
                    out=zj_bf,
                    in_=z_j[g * GM : (g + 1) * GM, :].rearrange(
                        "(c p) d -> p c d", p=P
                    ),
                )
                for jl in range(GCH):
                    j = g * GCH + jl
                    sq = work.tile([P, D], BF16, tag="sq")
                    nc.vector.scalar_tensor_tensor(
                        out=sq, in0=zj_bf[:, jl, :], scalar=1.0,
                        in1=zj_bf[:, jl, :],
                        op0=ALU.mult, op1=ALU.mult,
                        accum_out=nrm2_j[:, j : j + 1],
                    )
                gs = slice(g * GCH, (g + 1) * GCH)
                rsqrt_dve(nrm2_j[:, gs], t_j[:, gs], GCH)
                zjs = big.tile([P, GCH, D], BF16, tag=f"zjs{g}")
                for jl in range(GCH):
                    j = g * GCH + jl
                    nc.vector.tensor_scalar_mul(
                        out=zjs[:, jl, :],
                        in0=zj_bf[:, jl, :],
                        scalar1=t_j[:, j : j + 1],
                    )
                zjT = big.tile([P, GCH * KH, P], BF16, tag=f"zjT{g}")
                nc.sync.dma_start_transpose(
                    out=zjT, in_=zjs.rearrange("p c d -> p (c d)")
                )
                zjT_r.append(zjT.rearrange("do (c h) m -> do c h m", h=KH))

            # --- main compute: one [128, 1024] logits tile (one group's
            # m-range) + fused exp; pipelines at group granularity
            MW = 1024
            NSL = MW // 512
            lse_parts = stat.tile([P, NGROUP, NCHUNK], F32)

            def logits_tile(i, q):
                pt = psum.tile([P, MW], F32, tag="pt")
                for h in range(KH):
                    for jj in range(NSL):
                        c0 = jj * 4
                        nc.tensor.matmul(
                            pt[:, jj * 512 : (jj + 1) * 512],
                            lhsT=ziT_r[:, i, h, :],
                            rhs=zjT_r[q][:, c0 : c0 + 4, h, :],
                            start=(h == 0),
                            stop=(h == KH - 1),
                        )
                nc.scalar.activation(
                    out=pt,
                    in_=pt,
                    func=AF.Exp,
                    scale=s2[:, i : i + 1],
                    accum_out=lse_parts[:, q, i : i + 1],
                )

            zj_group(0)
            for i in range(NCHUNK):
                logits_tile(i, 0)
            zj_group(1)
            for i in range(NCHUNK):
                logits_tile(i, 1)

            # --- diagonal block: independent of main compute, slots into gaps
            zjd_bf = big.tile([P, NCHUNK, D], BF16)
            nc.gpsimd.dma_start(
                out=zjd_bf, in_=z_jd.rearrange("(c p) d -> p c d", p=P)
            )
            nrm2_d = stat.tile([P, NCHUNK], F32)
            for i in range(NCHUNK):
                sq = work.tile([P, D], BF16, tag="sq")
                nc.vector.scalar_tensor_tensor(
                    out=sq, in0=zjd_bf[:, i, :], scalar=1.0, in1=zjd_bf[:, i, :],
                    op0=ALU.mult, op1=ALU.mult,
                    accum_out=nrm2_d[:, i : i + 1],
                )
            t_d = stat.tile([P, NCHUNK], F32)
            rsqrt_dve(nrm2_d, t_d, NCHUNK)
            zjds = big.tile([P, NCHUNK, D], BF16)
            for i in range(NCHUNK):
                nc.vector.tensor_scalar_mul(
                    out=zjds[:, i, :], in0=zjd_bf[:, i, :], scalar1=t_d[:, i : i + 1]
                )
            dt = stat.tile([P, NCHUNK], F32)
            for i in range(NCHUNK):
                sq = work.tile([P, D], BF16, tag="sq")
                nc.vector.scalar_tensor_tensor(
                    out=sq, in0=zi_bf[:, i, :], scalar=1.0, in1=zjds[:, i, :],
                    op0=ALU.mult, op1=ALU.mult,
                    accum_out=dt[:, i : i + 1],
                )
            dg = stat.tile([P, NCHUNK], F32)
            nc.vector.tensor_mul(out=dg, in0=dt, in1=s2)

            zj_group(2)
            for i in range(NCHUNK):
                logits_tile(i, 2)
            zj_group(3)
            for i in range(NCHUNK):
                logits_tile(i, 3)

            # --- lse = ln(sum of the four quarter row-sums); contrib = lse - diag
            rs01 = stat.tile([P, NCHUNK], F32)
            nc.vector.tensor_add(
                out=rs01, in0=lse_parts[:, 0, :], in1=lse_parts[:, 1, :]
            )
            rs23 = stat.tile([P, NCHUNK], F32)
            nc.vector.tensor_add(
                out=rs23, in0=lse_parts[:, 2, :], in1=lse_parts[:, 3, :]
            )
            rs = stat.tile([P, NCHUNK], F32)
            nc.vector.tensor_add(out=rs, in0=rs01, in1=rs23)
            lse = stat.tile([P, NCHUNK], F32)
            nc.scalar.activation(out=lse, in_=rs, func=AF.Ln)
            contrib = stat.tile([P, NCHUNK], F32)
            nc.vector.tensor_sub(out=contrib, in0=lse, in1=dg)

            # --- partition reduction via ones-matmul: [1, 4] partials
            pt_fin = psum.tile([P, MW], F32, tag="pt")
            nc.tensor.matmul(
                pt_fin[:1, :NCHUNK], lhsT=ones, rhs=contrib, start=True, stop=True
            )
            osb = stat.tile([1, NCHUNK], F32)
            nc.vector.tensor_copy(out=osb, in_=pt_fin[:1, :NCHUNK])
            nc.sync.dma_start(out=out, in_=osb)

    nc.compile()
    return nc


_NC = None


def _get_nc():
    global _NC
    if _NC is None:
        _NC = build_nc()
    return _NC


def kernel(z_i: np.ndarray, z_j: np.ndarray, **_unused) -> np.ndarray:
    z_i = np.ascontiguousarray(z_i, dtype=np.float32)
    z_j = np.ascontiguousarray(z_j, dtype=np.float32)
    nc = _get_nc()
    in_maps = []
    for c in range(NCORES):
        sl = slice(c * NL, (c + 1) * NL)
        in_maps.append(
            {
                "z_i": z_i[sl],
                "z_j": z_j,
                "z_jd": z_j[sl],
            }
        )
    res = bass_utils.run_bass_kernel_spmd(
        nc, in_maps, core_ids=list(range(NCORES))
    )
    total = 0.0
    for c in range(NCORES):
        total += float(res.results[c]["out"].astype(np.float64).sum())
    return np.float32(total / N)


# revision 12
# speedup vs baseline: 2.7084x; 1.0149x over previous
"""Contrastive loss (SimCLR-style) on 8 TRN2 NeuronCores.

loss = -mean(diag(log_softmax(zi_n @ zj_n^T / T)))  with zi_n, zj_n L2-normalized,
N=4096, D=256, T=0.5.

Strategy (data-parallel over rows of z_i, z_j replicated):
  - core c gets rows [c*512, (c+1)*512) of z_i, the full z_j, and z_j's
    matching diagonal block as a separate small input.
  - cast to bf16 during load; row norms as one big multiply + one
    reduce per group; rsqrt on VectorE (bit-trick + 1 Newton step) so
    ScalarE's table set stays pinned to exp; row scaling on GpSimd (otherwise
    idle); one 3D-output xbar DMA transpose per group (SBUF->SBUF, no DRAM
    bounce); matmul in bf16 with f32 PSUM accumulate; fused exp+row-sum on
    ScalarE in place over PSUM (logits in [-2,2]: no max subtraction);
    diagonal via fused multiply+accumulate in normal layout; ones-matmul
    partition reduction.
  - z_j is processed in 4 pipelined groups; the logits loop runs
    half-m-range-outer so compute on groups 0-1 overlaps preprocessing of
    groups 2-3.
  - each core returns 4 partial sums of (lse[n] - logits[n,n]); host adds the
    32 values and divides by N.
"""

import numpy as np

import concourse.bass as bass
import concourse.bacc as bacc
import concourse.tile as tile
import concourse.bass_utils as bass_utils
from concourse import mybir

N = 4096
D = 256
NCORES = 8
NL = N // NCORES  # 512 local rows per core
P = 128
NCHUNK = NL // P  # 4 local row chunks
MCHUNK = N // P  # 32 zj chunks
NGROUP = 4  # zj processed in 4 groups of 8 chunks
GCH = MCHUNK // NGROUP  # 8 chunks per group
GM = GCH * P  # 1024 rows per group
KH = D // P  # 2 contraction halves
MAGIC = 0x5F3759DF

F32 = mybir.dt.float32
U32 = mybir.dt.uint32
BF16 = mybir.dt.bfloat16
AF = mybir.ActivationFunctionType
ALU = mybir.AluOpType
AX = mybir.AxisListType


def build_nc():
    nc = bacc.Bacc(
        "TRN2",
        target_bir_lowering=False,
        debug=False,
        enable_asserts=False,
    )
    z_i = nc.dram_tensor("z_i", (NL, D), F32, kind="ExternalInput").ap()
    z_j = nc.dram_tensor("z_j", (N, D), F32, kind="ExternalInput").ap()
    z_jd = nc.dram_tensor("z_jd", (NL, D), F32, kind="ExternalInput").ap()
    out = nc.dram_tensor("out", (1, NCHUNK), F32, kind="ExternalOutput").ap()

    with tile.TileContext(nc) as tc:
        with (
            tc.tile_pool(name="const", bufs=1) as const,
            tc.tile_pool(name="big", bufs=1) as big,
            tc.tile_pool(name="work", bufs=2) as work,
            tc.tile_pool(name="stat", bufs=1) as stat,
            tc.tile_pool(name="psum", bufs=4, space="PSUM") as psum,
        ):
            # --- dummy exp: force the exp ACT table set load at t=0
            dummy = const.tile([1, 1], F32)
            nc.vector.memset(dummy, 1.0)
            nc.scalar.activation(out=dummy, in_=dummy, func=AF.Exp)

            ones = const.tile([P, 1], F32)
            nc.vector.memset(ones, 1.0)
            magic = const.tile([P, GCH], U32)
            nc.vector.memset(magic, MAGIC)

            def rsqrt_dve(a, y, w):
                """y[:,:w] = 1/sqrt(a[:,:w]): quake seed + 1 Newton step."""
                au = a.bitcast(U32)
                yu = y.bitcast(U32)
                sh = work.tile([P, GCH], U32, tag="rsq_sh")
                nc.vector.tensor_scalar(
                    out=sh[:, :w], in0=au, scalar1=1, scalar2=None,
                    op0=ALU.logical_shift_right,
                )
                nc.vector.tensor_sub(out=yu, in0=magic[:, :w], in1=sh[:, :w])
                t1 = work.tile([P, GCH], F32, tag="rsq_t1")
                nc.vector.tensor_mul(out=t1[:, :w], in0=y, in1=y)
                nc.vector.tensor_mul(out=t1[:, :w], in0=t1[:, :w], in1=a)
                nc.vector.tensor_scalar(
                    out=t1[:, :w], in0=t1[:, :w], scalar1=-0.5, scalar2=1.5,
                    op0=ALU.mult, op1=ALU.add,
                )
                nc.vector.tensor_mul(out=y, in0=y, in1=t1[:, :w])

            # --- zi: load (cast bf16), transpose immediately, then norms
            zi_bf = big.tile([P, NCHUNK, D], BF16)
            nc.gpsimd.dma_start(
                out=zi_bf, in_=z_i.rearrange("(c p) d -> p c d", p=P)
            )
            ziT = big.tile([P, NCHUNK * KH, P], BF16)
            nc.scalar.dma_start_transpose(
                out=ziT, in_=zi_bf.rearrange("p c d -> p (c d)")
            )
            ziT_r = ziT.rearrange("do (i h) m -> do i h m", h=KH)

            nrm2_i = stat.tile([P, NCHUNK], F32)
            for i in range(NCHUNK):
                sq = work.tile([P, D], BF16, tag="sq")
                nc.vector.scalar_tensor_tensor(
                    out=sq, in0=zi_bf[:, i, :], scalar=1.0, in1=zi_bf[:, i, :],
                    op0=ALU.mult, op1=ALU.mult,
                    accum_out=nrm2_i[:, i : i + 1],
                )
            s2 = stat.tile([P, NCHUNK], F32)
            rsqrt_dve(nrm2_i, s2, NCHUNK)
            nc.vector.tensor_scalar(
                out=s2, in0=s2, scalar1=2.0, scalar2=None, op0=ALU.mult
            )

            # --- per-group zj: load -> norms -> rsqrt -> scale (GpSimd) ->
            #     one 3D xbar transpose
            nrm2_j = stat.tile([P, MCHUNK], F32)
            t_j = stat.tile([P, MCHUNK], F32)
            zjT_r = []

            def zj_group(g):
                zj_f = big.tile([P, GCH, D], F32, tag=f"zjf{g}")
                eng_ld = nc.sync if g % 2 == 0 else nc.scalar
                eng_ld.dma_start(
                    out=zj_f,
                    in_=z_j[g * GM : (g + 1) * GM, :].rearrange(
                        "(c p) d -> p c d", p=P
                    ),
                )
                for jl in range(GCH):
                    j = g * GCH + jl
                    sq = work.tile([P, D], BF16, tag="sq")
                    nc.vector.scalar_tensor_tensor(
                        out=sq, in0=zj_f[:, jl, :], scalar=1.0,
                        in1=zj_f[:, jl, :],
                        op0=ALU.mult, op1=ALU.mult,
                        accum_out=nrm2_j[:, j : j + 1],
                    )
                gs = slice(g * GCH, (g + 1) * GCH)
                rsqrt_dve(nrm2_j[:, gs], t_j[:, gs], GCH)
                zjs = big.tile([P, GCH, D], BF16, tag=f"zjs{g}")
                for jl in range(GCH):
                    j = g * GCH + jl
                    nc.vector.tensor_scalar_mul(
                        out=zjs[:, jl, :],
                        in0=zj_f[:, jl, :],
                        scalar1=t_j[:, j : j + 1],
                    )
                zjT = big.tile([P, GCH * KH, P], BF16, tag=f"zjT{g}")
                nc.sync.dma_start_transpose(
                    out=zjT, in_=zjs.rearrange("p c d -> p (c d)")
                )
                zjT_r.append(zjT.rearrange("do (c h) m -> do c h m", h=KH))

            # --- main compute: one [128, 1024] logits tile (one group's
            # m-range) + fused exp; pipelines at group granularity
            MW = 1024
            NSL = MW // 512
            lse_parts = stat.tile([P, NGROUP, NCHUNK], F32)

            def logits_tile(i, q):
                pt = psum.tile([P, MW], F32, tag="pt")
                for h in range(KH):
                    for jj in range(NSL):
                        c0 = jj * 4
                        nc.tensor.matmul(
                            pt[:, jj * 512 : (jj + 1) * 512],
                            lhsT=ziT_r[:, i, h, :],
                            rhs=zjT_r[q][:, c0 : c0 + 4, h, :],
                            start=(h == 0),
                            stop=(h == KH - 1),
                        )
                nc.scalar.activation(
                    out=pt,
                    in_=pt,
                    func=AF.Exp,
                    scale=s2[:, i : i + 1],
                    accum_out=lse_parts[:, q, i : i + 1],
                )

            zj_group(0)
            for i in range(NCHUNK):
                logits_tile(i, 0)
            zj_group(1)
            for i in range(NCHUNK):
                logits_tile(i, 1)

            # --- diagonal block: independent of main compute, slots into gaps
            zjd_f = big.tile([P, NCHUNK, D], F32)
            nc.sync.dma_start(
                out=zjd_f, in_=z_jd.rearrange("(c p) d -> p c d", p=P)
            )
            nrm2_d = stat.tile([P, NCHUNK], F32)
            for i in range(NCHUNK):
                sq = work.tile([P, D], BF16, tag="sq")
                nc.vector.scalar_tensor_tensor(
                    out=sq, in0=zjd_f[:, i, :], scalar=1.0, in1=zjd_f[:, i, :],
                    op0=ALU.mult, op1=ALU.mult,
                    accum_out=nrm2_d[:, i : i + 1],
                )
            t_d = stat.tile([P, NCHUNK], F32)
            rsqrt_dve(nrm2_d, t_d, NCHUNK)
            zjds = big.tile([P, NCHUNK, D], BF16)
            for i in range(NCHUNK):
                nc.vector.tensor_scalar_mul(
                    out=zjds[:, i, :], in0=zjd_f[:, i, :], scalar1=t_d[:, i : i + 1]
                )
            dt = stat.tile([P, NCHUNK], F32)
            for i in range(NCHUNK):
                sq = work.tile([P, D], BF16, tag="sq")
                nc.vector.scalar_tensor_tensor(
                    out=sq, in0=zi_bf[:, i, :], scalar=1.0, in1=zjds[:, i, :],
                    op0=ALU.mult, op1=ALU.mult,
                    accum_out=dt[:, i : i + 1],
                )
            dg = stat.tile([P, NCHUNK], F32)
            nc.vector.tensor_mul(out=dg, in0=dt, in1=s2)

            zj_group(2)
            for i in range(NCHUNK):
                logits_tile(i, 2)
            zj_group(3)
            for i in range(NCHUNK):
                logits_tile(i, 3)

            # --- lse = ln(sum of the four quarter row-sums); contrib = lse - diag
            rs01 = stat.tile([P, NCHUNK], F32)
            nc.vector.tensor_add(
                out=rs01, in0=lse_parts[:, 0, :], in1=lse_parts[:, 1, :]
            )
            rs23 = stat.tile([P, NCHUNK], F32)
            nc.vector.tensor_add(
                out=rs23, in0=lse_parts[:, 2, :], in1=lse_parts[:, 3, :]
            )
            rs = stat.tile([P, NCHUNK], F32)
            nc.vector.tensor_add(out=rs, in0=rs01, in1=rs23)
            lse = stat.tile([P, NCHUNK], F32)
            nc.scalar.activation(out=lse, in_=rs, func=AF.Ln)
            contrib = stat.tile([P, NCHUNK], F32)
            nc.vector.tensor_sub(out=contrib, in0=lse, in1=dg)

            # --- partition reduction via ones-matmul: [1, 4] partials
            pt_fin = psum.tile([P, MW], F32, tag="pt")
            nc.tensor.matmul(
                pt_fin[:1, :NCHUNK], lhsT=ones, rhs=contrib, start=True, stop=True
            )
            osb = stat.tile([1, NCHUNK], F32)
            nc.vector.tensor_copy(out=osb, in_=pt_fin[:1, :NCHUNK])
            nc.sync.dma_start(out=out, in_=osb)

    nc.compile()
    return nc


_NC = None


def _get_nc():
    global _NC
    if _NC is None:
        _NC = build_nc()
    return _NC


def kernel(z_i: np.ndarray, z_j: np.ndarray, **_unused) -> np.ndarray:
    z_i = np.ascontiguousarray(z_i, dtype=np.float32)
    z_j = np.ascontiguousarray(z_j, dtype=np.float32)
    nc = _get_nc()
    in_maps = []
    for c in range(NCORES):
        sl = slice(c * NL, (c + 1) * NL)
        in_maps.append(
            {
                "z_i": z_i[sl],
                "z_j": z_j,
                "z_jd": z_j[sl],
            }
        )
    res = bass_utils.run_bass_kernel_spmd(
        nc, in_maps, core_ids=list(range(NCORES))
    )
    total = 0.0
    for c in range(NCORES):
        total += float(res.results[c]["out"].astype(np.float64).sum())
    return np.float32(total / N)
